# revision 17
# baseline (speedup 1.0000x reference)
"""Trainium2 Bass kernel for BiDAF-style bidirectional attention.

Reference math (per batch b):
    sim[c,q]  = q[q]·wq + c[c]·wc + sum_e wm[e]*question[q,e]*context[c,e]
    c2q[c,:]  = softmax_q(sim[c,:]) @ question          # (C, E)
    q2c[:]    = softmax_c(max_q sim[c,:]) @ context     # (E,)
    out[c,:]  = [context | c2q | context*c2q | context*q2c]

Sharding: pure data parallel over batch (B=16 -> 2 batches per core x 8 cores).

v5 pipeline (fp16 compute, f32 I/O):
  - all HBM loads emitted up front on the sync HWDGE queue.
  - context transposes run on the DMA XBAR (dma_start_transpose, fp16,
    SBUF->SBUF, one [128,1024]->[128,8,128] op per 4-tile group) instead of
    64 PE transposes + 16 DVE PSUM->SBUF copies.  Groups are cast to fp16
    (scalar) and XBAR-transposed two rounds ahead of use.
  - pass A (per pair of 128-row tiles): fp16 sim matmul at N=130 with wc
    folded in as col 128 -> qw add + rowmax (DVE) -> exp (scalar).
  - pass B (lag 5): fp16 attention-weight transpose on the PE (4-deep PSUM
    ring) -> fp16 c2q matmul at N=258 whose col 256 is the softmax row-sum
    (ones column in the question) -> reciprocal (DVE) -> scalar copy-act
    rescale into cols E:2E -> ctx*c2q into cols 2E:3E: even pairs fused
    (c2q_raw * 1/s) * ctx straight out of PSUM on the DVE, odd pairs on
    gpsimd from the rescaled SBUF copy -> store cols E:3E on sync.
  - copy-through stores of cols 0:E ride the gpsimd SWDGE queue, spread
    over mid-kernel rounds as always-ready filler against store-supply
    starvation.  q2c epilogue per batch right after its pass A drains;
    ctx*q2c products split V/G; stage3 stores self-issued on SWDGE.
"""

import numpy as np

import concourse.bass as bass
import concourse.tile as tile
import concourse.mybir as mybir
from concourse import bacc
from concourse.bass_utils import run_bass_kernel_spmd
from concourse.masks import make_identity

B, C, Q, E = 16, 2048, 128, 256
NCORES = 8
BPC = B // NCORES          # batches per core
NT = C // 128              # context tiles per batch
NG = NT // 4               # groups of 4 tiles
NP = NT // 2               # pairs per batch
NGG = BPC * NG             # total groups per core
F32 = mybir.dt.float32
F16 = mybir.dt.float16
AX = mybir.AxisListType.X
EXP = mybir.ActivationFunctionType.Exp
CPY = mybir.ActivationFunctionType.Copy
MUL = mybir.AluOpType.mult
LAG = 5


class _Ctx:
    pass


def _body(tc, out_ext, ctx_in, q_in, wq_in, wc_in, wm_in):
    nc = tc.nc
    with (
        tc.tile_pool(name="singles", bufs=1) as singles,
        tc.tile_pool(name="stgp", bufs=NGG) as stgp,
        tc.tile_pool(name="xc16p", bufs=NGG) as xc16p,
        tc.tile_pool(name="xctp", bufs=4) as xctp,
        tc.tile_pool(name="qside", bufs=2) as qside,
        tc.tile_pool(name="work", bufs=4) as work,
        tc.tile_pool(name="pers", bufs=2 * NP) as pers,
        tc.tile_pool(name="statsp", bufs=2) as statsp,
        tc.tile_pool(name="ps_sim", bufs=2, space="PSUM") as ps_sim,
        tc.tile_pool(name="ps_pt", bufs=1, space="PSUM") as ps_pt,
        tc.tile_pool(name="ps_c2q", bufs=4, space="PSUM") as ps_c2q,
        tc.tile_pool(name="ps_misc", bufs=1, space="PSUM") as ps_misc,
    ):
        # ---- constants + params ------------------------------------------
        ident = singles.tile([128, 128], F32)
        make_identity(nc, ident)
        ident_h = singles.tile([128, 128], F16)
        make_identity(nc, ident_h)
        ones_r = singles.tile([1, 128], F32)
        nc.vector.memset(ones_r, 1.0)
        ones_c = singles.tile([128, 1], F32)
        nc.vector.memset(ones_c, 1.0)
        wq_sb = singles.tile([128, 2], F32)
        nc.sync.dma_start(out=wq_sb, in_=wq_in.rearrange("(j p) -> p j", p=128))
        wc_sb = singles.tile([128, 2], F32)
        nc.sync.dma_start(out=wc_sb, in_=wc_in.rearrange("(j p) -> p j", p=128))
        wm_sb = singles.tile([128, 2], F32)
        nc.sync.dma_start(out=wm_sb, in_=wm_in.rearrange("(j p) -> p j", p=128))

        # ---- all HBM loads up front --------------------------------------
        bs = []
        for b in range(BPC):
            st = _Ctx()
            bs.append(st)
            st.qm = qside.tile([128, E], F32, tag="qm", name="qm")
            nc.sync.dma_start(out=st.qm, in_=q_in[b])
        stgs = []      # indexed by global group gi = b*NG + g
        for b in range(BPC):
            for g in range(NG):
                stg = stgp.tile([128, 4, 4 * E], F32, tag="stg", name="stg")
                stgs.append(stg)
                nc.sync.dma_start(
                    out=stg[:, :, 0:E],
                    in_=ctx_in[b, g * 512 : (g + 1) * 512, :].rearrange(
                        "(t p) e -> p t e", p=128
                    ),
                )
        xcs = [None] * NGG    # fp16 shadows of the context, per group
        xcts = [None] * NGG   # XBAR-transposed fp16 context, per group

        def prep_group(gi, xbar_eng):
            # cast the group's context to fp16, then one XBAR DMA transpose:
            # xct[p, 2t+j, c] = xc16[c, t, j*128+p]
            xc16 = xc16p.tile([128, 4, E], F16, tag="xc16", name="xc16")
            nc.scalar.copy(out=xc16, in_=stgs[gi][:, :, 0:E])
            xcs[gi] = xc16
            xct = xctp.tile([128, 8, 128], F16, tag="xct", name="xct")
            xbar_eng.dma_start_transpose(out=xct, in_=xc16[:, :, :])
            xcts[gi] = xct

        # ---- copy-through of cols 0:E (SWDGE filler traffic) -------------
        def copythru(gi):
            b, g = gi // NG, gi % NG
            nc.gpsimd.dma_start(
                out=out_ext[b, g * 512 : (g + 1) * 512, 0:E].rearrange(
                    "(t p) f -> p t f", p=128
                ),
                in_=stgs[gi][:, :, 0:E],
            )

        # ---- question-side prep for both batches -------------------------
        for b in range(BPC):
            st = bs[b]
            qm = st.qm
            qmt_ps = ps_misc.tile([128, E], F32, tag="misc", name="qmt_ps")
            for j in range(2):
                nc.tensor.transpose(
                    qmt_ps[:, j * 128 : (j + 1) * 128],
                    qm[:, j * 128 : (j + 1) * 128],
                    ident,
                )
            qmt_sb = qside.tile([128, E], F32, tag="qmt", name="qmt_sb")
            nc.vector.tensor_copy(out=qmt_sb, in_=qmt_ps)
            # fp16 question for the c2q matmul; col 256 = 1 makes the
            # matmul's 257th output column the softmax row-sum.
            st.qm16 = qside.tile([128, E + 2], F16, tag="qm16", name="qm16")
            nc.vector.tensor_copy(out=st.qm16[:, 0:E], in_=qm)
            nc.vector.memset(st.qm16[:, E : E + 1], 1.0)
            nc.vector.memset(st.qm16[:, E + 1 : E + 2], 0.0)
            # rhs_aug[:, j, 0:128] = wm-chunk * QmT-chunk ; [:, j, 128] = wc
            # col 129 is zero pad to keep the PE output width even (N=130).
            st.rhs_aug = qside.tile([128, 2, 130], F16, tag="rhs_aug",
                                    name="rhs_aug")
            for j in range(2):
                nc.vector.tensor_scalar_mul(
                    st.rhs_aug[:, j, 0:128],
                    qmt_sb[:, j * 128 : (j + 1) * 128],
                    wm_sb[:, j : j + 1],
                )
                nc.vector.tensor_copy(
                    out=st.rhs_aug[:, j, 128:129], in_=wc_sb[:, j : j + 1]
                )
                nc.vector.memset(st.rhs_aug[:, j, 129:130], 0.0)
            qw_ps = ps_misc.tile([1, 128], F32, tag="misc", name="qw_ps")
            for j in range(2):
                nc.tensor.matmul(
                    qw_ps,
                    wq_sb[:, j : j + 1],
                    qmt_sb[:, j * 128 : (j + 1) * 128],
                    start=(j == 0),
                    stop=(j == 1),
                )
            qw_row = qside.tile([1, 128], F32, tag="qw_row", name="qw_row")
            nc.vector.tensor_copy(out=qw_row, in_=qw_ps)
            qwb_ps = ps_misc.tile([128, 128], F32, tag="misc", name="qwb_ps")
            nc.tensor.matmul(qwb_ps, ones_r, qw_row, start=True, stop=True)
            st.qwb2 = qside.tile([128, 2, 128], F32, tag="qwb2", name="qwb2")
            nc.vector.tensor_copy(out=st.qwb2[:, 0, :], in_=qwb_ps)
            nc.vector.tensor_copy(out=st.qwb2[:, 1, :], in_=qwb_ps)
            st.mstat = statsp.tile([128, NT], F32, tag="mstat", name="mstat")
            st.r_all = statsp.tile([128, NP, 2], F32, tag="r_all", name="r_all")
            st.p_sbs = {}

        # prefetch the first two groups (XBAR via the scalar HWDGE queue so
        # the sync queue's load burst isn't head-of-line blocked)
        prep_group(0, nc.scalar)
        prep_group(1, nc.scalar)

        # ---- pass A: sim + softmax stats + exp ---------------------------
        def pass_a(b, k):
            st = bs[b]
            kk = b * NP + k
            g, h = k // 2, k % 2
            gi = kk // 2
            if h == 0 and gi + 2 < NGG:
                prep_group(gi + 2, nc.sync)
            xct = xcts[gi]
            sim_ps = ps_sim.tile([128, 2, 130], F32, tag="sim", name="sim_ps")
            for i in range(2):
                for j in range(2):
                    nc.tensor.matmul(
                        sim_ps[:, i, :],
                        xct[:, 2 * (2 * h + i) + j, :],
                        st.rhs_aug[:, j, :],
                        start=(j == 0),
                        stop=(j == 1),
                    )
            nsim = work.tile([128, 2, 128], F32, tag="nsim", name="nsim")
            nm = work.tile([128, 2], F32, tag="nm", name="nm")
            nc.vector.tensor_add(nsim, sim_ps[:, :, 0:128], st.qwb2)
            nc.vector.reduce_max(out=nm, in_=nsim, axis=AX, negate=True)
            # mstat = cwc + rowmax = cwc - nm
            nc.vector.tensor_sub(
                st.mstat[:, 2 * k : 2 * k + 2], sim_ps[:, :, 128], nm
            )
            p_sb = pers.tile([128, 2, 128], F16, tag="p_sb", name="p_sb")
            for i in range(2):
                nc.scalar.activation(
                    out=p_sb[:, i, :],
                    in_=nsim[:, i, :],
                    func=EXP,
                    bias=nm[:, i : i + 1],
                    scale=1.0,
                )
            st.p_sbs[k] = p_sb

        # ---- pass B: c2q + ctx*c2q + store cols E:3E ---------------------
        def pass_b(b, k):
            st = bs[b]
            g, h = k // 2, k % 2
            gi = b * NG + g
            stg = stgs[gi]
            xc16 = xcs[gi]
            p_sb = st.p_sbs[k]
            pt_ps = st.pt_ring[:, k % 4, :, :]
            for i in range(2):
                nc.tensor.transpose(pt_ps[:, i, :], p_sb[:, i, :], ident_h)
            pt_sb = work.tile([128, 2, 128], F16, tag="pt_sb", name="pt_sb")
            nc.vector.tensor_copy(out=pt_sb, in_=pt_ps)
            for i in range(2):
                c2q_ps = ps_c2q.tile([128, E + 2], F32, tag="c2q",
                                     name="c2q_ps")
                nc.tensor.matmul(
                    c2q_ps, pt_sb[:, i, :], st.qm16, start=True, stop=True
                )
                nc.vector.reciprocal(
                    out=st.r_all[:, k, i : i + 1], in_=c2q_ps[:, E : E + 1]
                )
                nc.scalar.activation(
                    out=stg[:, 2 * h + i, E : 2 * E],
                    in_=c2q_ps[:, 0:E],
                    func=CPY,
                    scale=st.r_all[:, k, i : i + 1],
                )
                if k % 2 == 0:
                    # ctx*c2q = (c2q_raw * 1/s) * ctx, straight out of PSUM
                    # (DVE only: gpsimd has no PSUM access)
                    nc.vector.scalar_tensor_tensor(
                        out=stg[:, 2 * h + i, 2 * E : 3 * E],
                        in0=c2q_ps[:, 0:E],
                        scalar=st.r_all[:, k, i : i + 1],
                        in1=xc16[:, 2 * h + i, :],
                        op0=MUL,
                        op1=MUL,
                    )
            if k % 2 == 1:
                # odd pairs: SBUF-side product on gpsimd off the rescaled c2q
                nc.gpsimd.tensor_mul(
                    stg[:, 2 * h : 2 * h + 2, 2 * E : 3 * E],
                    stg[:, 2 * h : 2 * h + 2, E : 2 * E],
                    xc16[:, 2 * h : 2 * h + 2, :],
                )
            r0 = g * 512 + h * 256
            nc.sync.dma_start(
                out=out_ext[b, r0 : r0 + 256, E : 3 * E].rearrange(
                    "(t p) f -> p t f", p=128
                ),
                in_=stg[:, 2 * h : 2 * h + 2, E : 3 * E],
            )

        # ---- q2c epilogue: softmax over C, broadcast weights -------------
        def ep_pre(b):
            st = bs[b]
            mstat = st.mstat
            r1 = statsp.tile([128, 1], F32, tag="r1", name="r1")
            nc.vector.reduce_max(out=r1, in_=mstat, axis=AX)
            r1t_ps = ps_misc.tile([1, 128], F32, tag="misc", name="r1t_ps")
            nc.tensor.transpose(r1t_ps, r1, ident)
            neg_gmax = statsp.tile([1, 1], F32, tag="gmax", name="neg_gmax")
            nc.vector.reduce_max(
                out=neg_gmax, in_=r1t_ps, axis=AX, negate=True
            )
            ngb_ps = ps_misc.tile([128, 1], F32, tag="misc", name="ngb_ps")
            nc.tensor.matmul(ngb_ps, ones_r, neg_gmax, start=True, stop=True)
            ngb_sb = statsp.tile([128, 1], F32, tag="ngb", name="ngb_sb")
            nc.vector.tensor_copy(out=ngb_sb, in_=ngb_ps)
            st.e_sb = statsp.tile([128, NT], F16, tag="e_sb", name="e_sb")
            s_col = statsp.tile([128, 1], F32, tag="s_col", name="s_col")
            nc.scalar.activation(
                out=st.e_sb, in_=mstat, func=EXP, bias=ngb_sb, scale=1.0,
                accum_out=s_col,
            )
            tot_ps = ps_misc.tile([1, 1], F32, tag="misc", name="tot_ps")
            nc.tensor.matmul(tot_ps, s_col, ones_c, start=True, stop=True)
            st.rt_sb = statsp.tile([1, 1], F32, tag="rt", name="rt_sb")
            nc.vector.reciprocal(out=st.rt_sb, in_=tot_ps)

        def ep_q2c(b, half):
            st = bs[b]
            if half == 0:
                st.q2c_ps = ps_misc.tile([1, E], F32, tag="misc",
                                         name="q2c_ps")
            for t in range(half * NT // 2, (half + 1) * NT // 2):
                nc.tensor.matmul(
                    st.q2c_ps,
                    st.e_sb[:, t : t + 1],
                    xcs[b * NG + t // 4][:, t % 4, :],
                    start=(t == 0),
                    stop=(t == NT - 1),
                )

        def ep_fin(b):
            st = bs[b]
            q2c_sb = statsp.tile([1, E], F32, tag="q2c_sb", name="q2c_sb")
            nc.scalar.activation(
                out=q2c_sb, in_=st.q2c_ps, func=CPY, scale=st.rt_sb
            )
            q2cb_ps = ps_misc.tile([128, E], F32, tag="misc", name="q2cb_ps")
            nc.tensor.matmul(q2cb_ps, ones_r, q2c_sb, start=True, stop=True)
            st.q2cb16 = statsp.tile([128, 2, E], F16, tag="q2cb", name="q2cb16")
            nc.vector.tensor_copy(out=st.q2cb16[:, 0, :], in_=q2cb_ps)
            nc.vector.tensor_copy(out=st.q2cb16[:, 1, :], in_=q2cb_ps)

        # ---- ctx * q2c + store cols 3E:4E --------------------------------
        def stage3(b, g):
            st = bs[b]
            gi = b * NG + g
            stg = stgs[gi]
            xc16 = xcs[gi]
            for h in range(2):
                # mid-kernel (batch 0) stage3 runs fully on gpsimd (DVE is
                # the loaded engine there); the tail (batch 1) splits V/G.
                eng = nc.vector if (b == 1 and h == 0) else nc.gpsimd
                eng.tensor_mul(
                    stg[:, 2 * h : 2 * h + 2, 3 * E : 4 * E],
                    xc16[:, 2 * h : 2 * h + 2, :],
                    st.q2cb16,
                )
            nc.gpsimd.dma_start(
                out=out_ext[
                    b, g * 512 : (g + 1) * 512, 3 * E : 4 * E
                ].rearrange("(t p) f -> p t f", p=128),
                in_=stg[:, :, 3 * E : 4 * E],
            )

        # ---- schedule ----------------------------------------------------
        for b in range(BPC):
            bs[b].pt_ring = ps_pt.tile(
                [128, 4, 2, 128], F16, tag="pt", name="pt_ring"
            )
        # Modulo schedule over global pair index kk = b*NP + k.  Pass B lags
        # pass A by LAG pairs; pass B is emitted first inside each round
        # (its inputs are oldest, hence ready).  The q2c epilogue chain for
        # each batch starts right after that batch's pass A drains and is
        # split into small pieces so it never parks mid-queue in front of
        # ready pass-B work.  Copy-through stores fill rounds 8..15.
        TOT = BPC * NP
        for r in range(TOT + LAG + NG + 1):
            if r >= LAG and r - LAG < TOT:
                kk = r - LAG
                pass_b(kk // NP, kk % NP)
            if r < TOT:
                pass_a(r // NP, r % NP)
            if NP <= r < NP + NGG:
                copythru(r - NP)
            if r == NP:
                ep_pre(0)
            elif r == NP + 1:
                ep_q2c(0, 0)
            elif r == NP + 2:
                ep_q2c(0, 1)
            elif r == NP + 3:
                ep_fin(0)
            elif NP + 4 <= r < NP + 4 + 2 * NG and (r - NP) % 2 == 0:
                stage3(0, (r - (NP + 4)) // 2)
            if r == TOT:
                ep_pre(1)
            elif r == TOT + 1:
                ep_q2c(1, 0)
                ep_q2c(1, 1)
            elif r == TOT + 2:
                ep_fin(1)
            elif TOT + 3 <= r < TOT + 3 + NG:
                stage3(1, r - (TOT + 3))


_NC_CACHE = None


def _build():
    global _NC_CACHE
    if _NC_CACHE is not None:
        return _NC_CACHE
    nc = bacc.Bacc(
        "TRN2", target_bir_lowering=False, debug=False, num_devices=NCORES
    )
    ctx_in = nc.dram_tensor("context", [BPC, C, E], F32, kind="ExternalInput").ap()
    q_in = nc.dram_tensor("question", [BPC, Q, E], F32, kind="ExternalInput").ap()
    wq_in = nc.dram_tensor("w_question", [E], F32, kind="ExternalInput").ap()
    wc_in = nc.dram_tensor("w_context", [E], F32, kind="ExternalInput").ap()
    wm_in = nc.dram_tensor("w_multiple", [E], F32, kind="ExternalInput").ap()
    out_ext = nc.dram_tensor("out", [BPC, C, 4 * E], F32, kind="ExternalOutput").ap()
    with tile.TileContext(nc) as tc:
        _body(tc, out_ext, ctx_in, q_in, wq_in, wc_in, wm_in)
    nc.compile()
    _NC_CACHE = nc
    return nc


def _run(inputs, trace=False, **kw):
    nc = _build()
    context = np.ascontiguousarray(np.asarray(inputs["context"], dtype=np.float32))
    question = np.ascontiguousarray(np.asarray(inputs["question"], dtype=np.float32))
    wq = np.ascontiguousarray(np.asarray(inputs["w_question"], dtype=np.float32))
    wc = np.ascontiguousarray(np.asarray(inputs["w_context"], dtype=np.float32))
    wm = np.ascontiguousarray(np.asarray(inputs["w_multiple"], dtype=np.float32))
    in_maps = []
    for i in range(NCORES):
        sl = slice(i * BPC, (i + 1) * BPC)
        in_maps.append(
            {
                "context": context[sl],
                "question": question[sl],
                "w_question": wq,
                "w_context": wc,
                "w_multiple": wm,
            }
        )
    res = run_bass_kernel_spmd(
        nc, in_maps, core_ids=list(range(NCORES)), trace=trace, **kw
    )
    out = np.concatenate([res.results[i]["out"] for i in range(NCORES)], axis=0)
    return out, res


def kernel(**inputs):
    try:
        out, _ = _run(inputs, trace=False)
    except Exception:
        # transient device errors (e.g. a wedged core from a prior run)
        # usually clear on retry
        out, _ = _run(inputs, trace=False)
    return out


# revision 20
# speedup vs baseline: 1.0628x; 1.0628x over previous
"""Trainium2 Bass kernel for BiDAF-style bidirectional attention.

Reference math (per batch b):
    sim[c,q]  = q[q]·wq + c[c]·wc + sum_e wm[e]*question[q,e]*context[c,e]
    c2q[c,:]  = softmax_q(sim[c,:]) @ question          # (C, E)
    q2c[:]    = softmax_c(max_q sim[c,:]) @ context     # (E,)
    out[c,:]  = [context | c2q | context*c2q | context*q2c]

Sharding: pure data parallel over batch (B=16 -> 2 batches per core x 8 cores).

v2 pipeline (fp16 compute, f32 I/O):
  - all loads emitted up front on the sync HWDGE queue, followed by the
    copy-through stores of cols 0:E (verbatim context); the queue then has
    ~24us of transfer queued before the first dependent store wait.
  - pass A (per pair of 128-row context tiles): fp16 shadow cast of the
    group (gpsimd) -> PE transpose (fp16) -> fp16 sim matmul at N=130 with
    wc folded in as col 128 -> fused add+rowmax on DVE (tensor_tensor_
    reduce emits -(sim+qw) and its min = -rowmax) -> exp on scalar with
    accum_out giving the softmax row-sum for free.
  - pass B (lag 3): fp16 attention-weight transpose (4-deep PSUM ring) ->
    fp16 c2q matmul (N=256, no ones column needed) -> reciprocal of the
    stashed row-sums -> scalar copy-act rescale into cols E:2E -> fused
    (c2q_raw * 1/s) * ctx product straight out of PSUM into cols 2E:3E via
    DVE scalar_tensor_tensor -> store cols E:3E on the sync queue.
  - q2c epilogue per batch right after its pass A drains; ctx*q2c products
    (fp16 x fp16) on gpsimd, stores on the scalar HWDGE queue.
"""

import numpy as np

import concourse.bass as bass
import concourse.tile as tile
import concourse.mybir as mybir
from concourse import bacc
from concourse.bass_utils import run_bass_kernel_spmd
from concourse.masks import make_identity

B, C, Q, E = 16, 2048, 128, 256
NCORES = 8
BPC = B // NCORES          # batches per core
NT = C // 128              # context tiles per batch
NG = NT // 4               # groups of 4 tiles
NP = NT // 2               # pairs per batch
F32 = mybir.dt.float32
F16 = mybir.dt.float16
AX = mybir.AxisListType.X
EXP = mybir.ActivationFunctionType.Exp
CPY = mybir.ActivationFunctionType.Copy
MUL = mybir.AluOpType.mult
ADD = mybir.AluOpType.add
MIN = mybir.AluOpType.min
LAG = 5
import os
# tensor_tensor_reduce crashes/hangs on HW in this usage (in0=PSUM,
# scale=-1, op1=min) — keep the 2-op fallback. scalar_tensor_tensor works.
USE_TTR = os.environ.get("K_TTR", "0") == "1"
USE_STT = os.environ.get("K_STT", "1") == "1"


class _Ctx:
    pass


def _body(tc, out_ext, ctx_in, q_in, wq_in, wc_in, wm_in):
    nc = tc.nc
    with (
        tc.tile_pool(name="singles", bufs=1) as singles,
        tc.tile_pool(name="stgp", bufs=BPC * NG) as stgp,
        tc.tile_pool(name="xc16p", bufs=BPC * NG) as xc16p,
        tc.tile_pool(name="qside", bufs=2) as qside,
        tc.tile_pool(name="work", bufs=4) as work,
        tc.tile_pool(name="pers", bufs=2 * NP) as pers,
        tc.tile_pool(name="statsp", bufs=2) as statsp,
        tc.tile_pool(name="ps_xct", bufs=2, space="PSUM") as ps_xct,
        tc.tile_pool(name="ps_sim", bufs=2, space="PSUM") as ps_sim,
        tc.tile_pool(name="ps_pt", bufs=1, space="PSUM") as ps_pt,
        tc.tile_pool(name="ps_c2q", bufs=1, space="PSUM") as ps_c2q,
        tc.tile_pool(name="ps_misc", bufs=1, space="PSUM") as ps_misc,
    ):
        # ---- constants + params ------------------------------------------
        ident = singles.tile([128, 128], F32)
        make_identity(nc, ident)
        ident_h = singles.tile([128, 128], F16)
        make_identity(nc, ident_h)
        ones_r = singles.tile([1, 128], F32)
        nc.vector.memset(ones_r, 1.0)
        ones_c = singles.tile([128, 1], F32)
        nc.vector.memset(ones_c, 1.0)
        wq_sb = singles.tile([128, 2], F32)
        nc.sync.dma_start(out=wq_sb, in_=wq_in.rearrange("(j p) -> p j", p=128))
        wc_sb = singles.tile([128, 2], F32)
        nc.sync.dma_start(out=wc_sb, in_=wc_in.rearrange("(j p) -> p j", p=128))
        wm_sb = singles.tile([128, 2], F32)
        nc.sync.dma_start(out=wm_sb, in_=wm_in.rearrange("(j p) -> p j", p=128))

        # ---- all loads up front + copy-through of cols 0:E ---------------
        bs = []
        for b in range(BPC):
            st = _Ctx()
            bs.append(st)
            st.qm = qside.tile([128, E], F32, tag="qm", name="qm")
            nc.sync.dma_start(out=st.qm, in_=q_in[b])
        for b in range(BPC):
            st = bs[b]
            st.stgs = []
            for g in range(NG):
                stg = stgp.tile([128, 4, 4 * E], F32, tag="stg", name="stg")
                st.stgs.append(stg)
                nc.sync.dma_start(
                    out=stg[:, :, 0:E],
                    in_=ctx_in[b, g * 512 : (g + 1) * 512, :].rearrange(
                        "(t p) e -> p t e", p=128
                    ),
                )
        xcs = [None] * (BPC * NG)   # fp16 context shadows, global group idx

        def cast_group(gi):
            xc16 = xc16p.tile([128, 4, E], F16, tag="xc16", name="xc16")
            nc.scalar.copy(out=xc16, in_=bs[gi // NG].stgs[gi % NG][:, :, 0:E])
            xcs[gi] = xc16

        def copythru(b, g):
            # out[:, :, 0:E] is exactly the context; issued on the gpsimd
            # SWDGE queue so it drains independently of the sync queue --
            # always-ready filler traffic for any store-readiness stall.
            nc.gpsimd.dma_start(
                out=out_ext[b, g * 512 : (g + 1) * 512, 0:E].rearrange(
                    "(t p) f -> p t f", p=128
                ),
                in_=bs[b].stgs[g][:, :, 0:E],
            )

        # ---- question-side prep for both batches -------------------------
        for b in range(BPC):
            st = bs[b]
            qm = st.qm
            qmt_ps = ps_xct.tile([128, E], F32, tag="xct", name="qmt_ps")
            for j in range(2):
                nc.tensor.transpose(
                    qmt_ps[:, j * 128 : (j + 1) * 128],
                    qm[:, j * 128 : (j + 1) * 128],
                    ident,
                )
            qmt_sb = qside.tile([128, E], F32, tag="qmt", name="qmt_sb")
            nc.vector.tensor_copy(out=qmt_sb, in_=qmt_ps)
            # fp16 question for the c2q matmul
            st.qm16 = qside.tile([128, E], F16, tag="qm16", name="qm16")
            nc.vector.tensor_copy(out=st.qm16, in_=qm)
            # rhs_aug[:, j, 0:128] = wm-chunk * QmT-chunk ; [:, j, 128] = wc
            # col 129 is zero pad to keep the PE output width even (N=130).
            st.rhs_aug = qside.tile([128, 2, 130], F16, tag="rhs_aug",
                                    name="rhs_aug")
            for j in range(2):
                nc.vector.tensor_scalar_mul(
                    st.rhs_aug[:, j, 0:128],
                    qmt_sb[:, j * 128 : (j + 1) * 128],
                    wm_sb[:, j : j + 1],
                )
                nc.vector.tensor_copy(
                    out=st.rhs_aug[:, j, 128:129], in_=wc_sb[:, j : j + 1]
                )
                nc.vector.memset(st.rhs_aug[:, j, 129:130], 0.0)
            qw_ps = ps_misc.tile([1, 128], F32, tag="misc", name="qw_ps")
            for j in range(2):
                nc.tensor.matmul(
                    qw_ps,
                    wq_sb[:, j : j + 1],
                    qmt_sb[:, j * 128 : (j + 1) * 128],
                    start=(j == 0),
                    stop=(j == 1),
                )
            qw_row = qside.tile([1, 128], F32, tag="qw_row", name="qw_row")
            nc.vector.tensor_copy(out=qw_row, in_=qw_ps)
            qwb_ps = ps_misc.tile([128, 128], F32, tag="misc", name="qwb_ps")
            nc.tensor.matmul(qwb_ps, ones_r, qw_row, start=True, stop=True)
            st.qwb2 = qside.tile([128, 2, 128], F32, tag="qwb2", name="qwb2")
            nc.vector.tensor_copy(out=st.qwb2[:, 0, :], in_=qwb_ps)
            nc.vector.tensor_copy(out=st.qwb2[:, 1, :], in_=qwb_ps)
            st.mstat = statsp.tile([128, NT], F32, tag="mstat", name="mstat")
            st.s_all = statsp.tile([128, NP, 2], F32, tag="s_all", name="s_all")
            st.r_all = statsp.tile([128, NP, 2], F32, tag="r_all", name="r_all")
            st.p_sbs = {}

        # prefetch the first group's fp16 shadow before the round loop
        cast_group(0)

        # ---- pass A: sim + softmax stats + exp ---------------------------
        def pass_a(b, k):
            st = bs[b]
            g, h = k // 2, k % 2
            stg = st.stgs[g]
            gi = b * NG + g
            if h == 0 and gi + 1 < BPC * NG:
                # prefetch the NEXT group's cast so this round's PE
                # transposes never wait on the scalar queue
                cast_group(gi + 1)
            xc16 = xcs[gi]
            xct_ps = ps_xct.tile([128, 2, E], F16, tag="xct", name="xct_ps")
            for i in range(2):
                for j in range(2):
                    nc.tensor.transpose(
                        xct_ps[:, i, j * 128 : (j + 1) * 128],
                        xc16[:, 2 * h + i, j * 128 : (j + 1) * 128],
                        ident_h,
                    )
            xct_sb = work.tile([128, 2, E], F16, tag="xct_sb", name="xct_sb")
            nc.vector.tensor_copy(out=xct_sb, in_=xct_ps)
            sim_ps = ps_sim.tile([128, 2, 130], F32, tag="sim", name="sim_ps")
            for i in range(2):
                for j in range(2):
                    nc.tensor.matmul(
                        sim_ps[:, i, :],
                        xct_sb[:, i, j * 128 : (j + 1) * 128],
                        st.rhs_aug[:, j, :],
                        start=(j == 0),
                        stop=(j == 1),
                    )
            # nsim = -(sim + qw); nm = min(nsim) = -rowmax  (fused on DVE)
            nsim = work.tile([128, 2, 128], F32, tag="nsim", name="nsim")
            nm = work.tile([128, 2], F32, tag="nm", name="nm")
            if USE_TTR:
                for i in range(2):
                    nc.vector.tensor_tensor_reduce(
                        out=nsim[:, i, :],
                        in0=sim_ps[:, i, 0:128],
                        in1=st.qwb2[:, i, :],
                        scale=-1.0,
                        scalar=3.0e38,
                        op0=ADD,
                        op1=MIN,
                        accum_out=nm[:, i : i + 1],
                    )
            else:
                # fallback: sim_in kept positive, nm = -rowmax via negate;
                # the exp then runs with scale=+1 (v1 style).
                nc.vector.tensor_add(nsim, sim_ps[:, :, 0:128], st.qwb2)
                nc.vector.reduce_max(out=nm, in_=nsim, axis=AX, negate=True)
            # mstat = cwc + rowmax = cwc - nm
            nc.vector.tensor_sub(
                st.mstat[:, 2 * k : 2 * k + 2], sim_ps[:, :, 128], nm
            )
            p_sb = pers.tile([128, 2, 128], F16, tag="p_sb", name="p_sb")
            for i in range(2):
                # p = exp(-nsim + nm) = exp(sim + qw - rowmax); row-sum freed
                # into s_all by the activation accumulator.
                nc.scalar.activation(
                    out=p_sb[:, i, :],
                    in_=nsim[:, i, :],
                    func=EXP,
                    bias=nm[:, i : i + 1],
                    scale=-1.0 if USE_TTR else 1.0,
                    accum_out=st.s_all[:, k, i : i + 1],
                )
            st.p_sbs[k] = p_sb

        # ---- pass B: c2q + ctx*c2q + store cols E:3E ---------------------
        def pass_b(b, k):
            st = bs[b]
            g, h = k // 2, k % 2
            stg = st.stgs[g]
            xc16 = xcs[b * NG + g]
            p_sb = st.p_sbs[k]
            pt_ps = st.pt_ring[:, k % 4, :, :]
            for i in range(2):
                nc.tensor.transpose(pt_ps[:, i, :], p_sb[:, i, :], ident_h)
            pt_sb = work.tile([128, 2, 128], F16, tag="pt_sb", name="pt_sb")
            nc.vector.tensor_copy(out=pt_sb, in_=pt_ps)
            nc.vector.reciprocal(
                out=st.r_all[:, k, :], in_=st.s_all[:, k, :]
            )
            for i in range(2):
                c2q_ps = st.c2q_ring[:, k % 2, i, :]
                nc.tensor.matmul(
                    c2q_ps, pt_sb[:, i, :], st.qm16, start=True, stop=True
                )
                nc.scalar.activation(
                    out=stg[:, 2 * h + i, E : 2 * E],
                    in_=c2q_ps,
                    func=CPY,
                    scale=st.r_all[:, k, i : i + 1],
                )
                if k % 2 == 0 and USE_STT:
                    # ctx*c2q = (c2q_raw * 1/s) * ctx, straight out of PSUM
                    # (DVE only: gpsimd has no PSUM access)
                    nc.vector.scalar_tensor_tensor(
                        out=stg[:, 2 * h + i, 2 * E : 3 * E],
                        in0=c2q_ps,
                        scalar=st.r_all[:, k, i : i + 1],
                        in1=xc16[:, 2 * h + i, :],
                        op0=MUL,
                        op1=MUL,
                    )
                elif k % 2 == 0:
                    nc.vector.tensor_mul(
                        stg[:, 2 * h + i, 2 * E : 3 * E],
                        stg[:, 2 * h + i, E : 2 * E],
                        xc16[:, 2 * h + i, :],
                    )
            if k % 2 == 1:
                # odd pairs: SBUF-side product on gpsimd off the rescaled c2q
                nc.gpsimd.tensor_mul(
                    stg[:, 2 * h : 2 * h + 2, 2 * E : 3 * E],
                    stg[:, 2 * h : 2 * h + 2, E : 2 * E],
                    xc16[:, 2 * h : 2 * h + 2, :],
                )
            r0 = g * 512 + h * 256
            nc.sync.dma_start(
                out=out_ext[b, r0 : r0 + 256, E : 3 * E].rearrange(
                    "(t p) f -> p t f", p=128
                ),
                in_=stg[:, 2 * h : 2 * h + 2, E : 3 * E],
            )

        # ---- q2c epilogue: softmax over C, broadcast weights -------------
        def ep_pre(b):
            st = bs[b]
            mstat = st.mstat
            r1 = statsp.tile([128, 1], F32, tag="r1", name="r1")
            nc.vector.reduce_max(out=r1, in_=mstat, axis=AX)
            r1t_ps = ps_misc.tile([1, 128], F32, tag="misc", name="r1t_ps")
            nc.tensor.transpose(r1t_ps, r1, ident)
            neg_gmax = statsp.tile([1, 1], F32, tag="gmax", name="neg_gmax")
            nc.vector.reduce_max(
                out=neg_gmax, in_=r1t_ps, axis=AX, negate=True
            )
            ngb_ps = ps_misc.tile([128, 1], F32, tag="misc", name="ngb_ps")
            nc.tensor.matmul(ngb_ps, ones_r, neg_gmax, start=True, stop=True)
            ngb_sb = statsp.tile([128, 1], F32, tag="ngb", name="ngb_sb")
            nc.vector.tensor_copy(out=ngb_sb, in_=ngb_ps)
            st.e_sb = statsp.tile([128, NT], F16, tag="e_sb", name="e_sb")
            s_col = statsp.tile([128, 1], F32, tag="s_col", name="s_col")
            nc.scalar.activation(
                out=st.e_sb, in_=mstat, func=EXP, bias=ngb_sb, scale=1.0,
                accum_out=s_col,
            )
            tot_ps = ps_misc.tile([1, 1], F32, tag="misc", name="tot_ps")
            nc.tensor.matmul(tot_ps, s_col, ones_c, start=True, stop=True)
            st.rt_sb = statsp.tile([1, 1], F32, tag="rt", name="rt_sb")
            nc.vector.reciprocal(out=st.rt_sb, in_=tot_ps)

        def ep_q2c(b, half):
            st = bs[b]
            if half == 0:
                st.q2c_ps = ps_misc.tile([1, E], F32, tag="misc",
                                         name="q2c_ps")
            for t in range(half * NT // 2, (half + 1) * NT // 2):
                nc.tensor.matmul(
                    st.q2c_ps,
                    st.e_sb[:, t : t + 1],
                    xcs[b * NG + t // 4][:, t % 4, :],
                    start=(t == 0),
                    stop=(t == NT - 1),
                )

        def ep_fin(b):
            st = bs[b]
            q2c_sb = statsp.tile([1, E], F32, tag="q2c_sb", name="q2c_sb")
            nc.scalar.activation(
                out=q2c_sb, in_=st.q2c_ps, func=CPY, scale=st.rt_sb
            )
            q2cb_ps = ps_misc.tile([128, E], F32, tag="misc", name="q2cb_ps")
            nc.tensor.matmul(q2cb_ps, ones_r, q2c_sb, start=True, stop=True)
            st.q2cb16 = statsp.tile([128, 2, E], F16, tag="q2cb", name="q2cb16")
            nc.vector.tensor_copy(out=st.q2cb16[:, 0, :], in_=q2cb_ps)
            nc.vector.tensor_copy(out=st.q2cb16[:, 1, :], in_=q2cb_ps)

        # ---- ctx * q2c + store cols 3E:4E --------------------------------
        def stage3(b, g):
            st = bs[b]
            stg = st.stgs[g]
            xc16 = xcs[b * NG + g]
            for h in range(2):
                # mid-kernel (batch 0) stage3 runs fully on gpsimd (DVE is
                # the loaded engine there); the tail (batch 1) splits V/G.
                eng = nc.vector if (b == 1 and h == 0) else nc.gpsimd
                eng.tensor_mul(
                    stg[:, 2 * h : 2 * h + 2, 3 * E : 4 * E],
                    xc16[:, 2 * h : 2 * h + 2, :],
                    st.q2cb16,
                )
            nc.gpsimd.dma_start(
                out=out_ext[
                    b, g * 512 : (g + 1) * 512, 3 * E : 4 * E
                ].rearrange("(t p) f -> p t f", p=128),
                in_=stg[:, :, 3 * E : 4 * E],
            )

        # ---- schedule ----------------------------------------------------
        for b in range(BPC):
            bs[b].pt_ring = ps_pt.tile(
                [128, 4, 2, 128], F16, tag="pt", name="pt_ring"
            )
            bs[b].c2q_ring = ps_c2q.tile(
                [128, 2, 2, E], F32, tag="c2q", name="c2q_ring"
            )
        # Modulo schedule over global pair index kk = b*NP + k.  Pass B lags
        # pass A by LAG pairs; pass B is emitted first inside each round
        # (its inputs are oldest, hence ready).  The q2c epilogue chain for
        # each batch starts right after that batch's pass A drains and is
        # split into small pieces so it never parks mid-queue in front of
        # ready pass-B work.
        TOT = BPC * NP
        for r in range(TOT + LAG + NG + 1):
            if r >= LAG and r - LAG < TOT:
                kk = r - LAG
                pass_b(kk // NP, kk % NP)
            if r < TOT:
                pass_a(r // NP, r % NP)
            if 1 <= r <= BPC * NG:
                g = r - 1
                copythru(g // NG, g % NG)
            if r == NP:
                ep_pre(0)
            elif r == NP + 1:
                ep_q2c(0, 0)
            elif r == NP + 2:
                ep_q2c(0, 1)
            elif r == NP + 3:
                ep_fin(0)
            elif NP + 4 <= r < NP + 4 + 2 * NG and (r - NP) % 2 == 0:
                stage3(0, (r - (NP + 4)) // 2)
            if r == TOT:
                ep_pre(1)
            elif r == TOT + 1:
                ep_q2c(1, 0)
                ep_q2c(1, 1)
            elif r == TOT + 2:
                ep_fin(1)
            elif TOT + 3 <= r < TOT + 3 + NG:
                stage3(1, r - (TOT + 3))


_NC_CACHE = None


def _build():
    global _NC_CACHE
    if _NC_CACHE is not None:
        return _NC_CACHE
    nc = bacc.Bacc(
        "TRN2", target_bir_lowering=False, debug=False, num_devices=NCORES
    )
    ctx_in = nc.dram_tensor("context", [BPC, C, E], F32, kind="ExternalInput").ap()
    q_in = nc.dram_tensor("question", [BPC, Q, E], F32, kind="ExternalInput").ap()
    wq_in = nc.dram_tensor("w_question", [E], F32, kind="ExternalInput").ap()
    wc_in = nc.dram_tensor("w_context", [E], F32, kind="ExternalInput").ap()
    wm_in = nc.dram_tensor("w_multiple", [E], F32, kind="ExternalInput").ap()
    out_ext = nc.dram_tensor("out", [BPC, C, 4 * E], F32, kind="ExternalOutput").ap()
    with tile.TileContext(nc) as tc:
        _body(tc, out_ext, ctx_in, q_in, wq_in, wc_in, wm_in)
    nc.compile()
    _NC_CACHE = nc
    return nc


def _run(inputs, trace=False, **kw):
    nc = _build()
    context = np.ascontiguousarray(np.asarray(inputs["context"], dtype=np.float32))
    question = np.ascontiguousarray(np.asarray(inputs["question"], dtype=np.float32))
    wq = np.ascontiguousarray(np.asarray(inputs["w_question"], dtype=np.float32))
    wc = np.ascontiguousarray(np.asarray(inputs["w_context"], dtype=np.float32))
    wm = np.ascontiguousarray(np.asarray(inputs["w_multiple"], dtype=np.float32))
    in_maps = []
    for i in range(NCORES):
        sl = slice(i * BPC, (i + 1) * BPC)
        in_maps.append(
            {
                "context": context[sl],
                "question": question[sl],
                "w_question": wq,
                "w_context": wc,
                "w_multiple": wm,
            }
        )
    res = run_bass_kernel_spmd(
        nc, in_maps, core_ids=list(range(NCORES)), trace=trace, **kw
    )
    out = np.concatenate([res.results[i]["out"] for i in range(NCORES)], axis=0)
    return out, res


def kernel(**inputs):
    try:
        out, _ = _run(inputs, trace=False)
    except Exception:
        # transient device errors (e.g. a wedged core from a prior run)
        # usually clear on retry
        out, _ = _run(inputs, trace=False)
    return out


# revision 21
# speedup vs baseline: 1.2316x; 1.1589x over previous
"""Trainium2 Bass kernel for BiDAF-style bidirectional attention.

Reference math (per batch b):
    sim[c,q]  = q[q]·wq + c[c]·wc + sum_e wm[e]*question[q,e]*context[c,e]
    c2q[c,:]  = softmax_q(sim[c,:]) @ question          # (C, E)
    q2c[:]    = softmax_c(max_q sim[c,:]) @ context     # (E,)
    out[c,:]  = [context | c2q | context*c2q | context*q2c]

Sharding: pure data parallel over batch (B=16 -> 2 batches per core x 8 cores).

v2 pipeline (fp16 compute, f32 I/O):
  - all loads emitted up front on the sync HWDGE queue, followed by the
    copy-through stores of cols 0:E (verbatim context); the queue then has
    ~24us of transfer queued before the first dependent store wait.
  - pass A (per pair of 128-row context tiles): fp16 shadow cast of the
    group (gpsimd) -> PE transpose (fp16) -> fp16 sim matmul at N=130 with
    wc folded in as col 128 -> fused add+rowmax on DVE (tensor_tensor_
    reduce emits -(sim+qw) and its min = -rowmax) -> exp on scalar with
    accum_out giving the softmax row-sum for free.
  - pass B (lag 3): fp16 attention-weight transpose (4-deep PSUM ring) ->
    fp16 c2q matmul (N=256, no ones column needed) -> reciprocal of the
    stashed row-sums -> scalar copy-act rescale into cols E:2E -> fused
    (c2q_raw * 1/s) * ctx product straight out of PSUM into cols 2E:3E via
    DVE scalar_tensor_tensor -> store cols E:3E on the sync queue.
  - q2c epilogue per batch right after its pass A drains; ctx*q2c products
    (fp16 x fp16) on gpsimd, stores on the scalar HWDGE queue.
"""

import numpy as np

import concourse.bass as bass
import concourse.tile as tile
import concourse.mybir as mybir
from concourse import bacc
from concourse.bass_utils import run_bass_kernel_spmd
from concourse.masks import make_identity

B, C, Q, E = 16, 2048, 128, 256
NCORES = 8
BPC = B // NCORES          # batches per core
NT = C // 128              # context tiles per batch
NG = NT // 4               # groups of 4 tiles
NP = NT // 2               # pairs per batch
F32 = mybir.dt.float32
F16 = mybir.dt.float16
AX = mybir.AxisListType.X
EXP = mybir.ActivationFunctionType.Exp
CPY = mybir.ActivationFunctionType.Copy
MUL = mybir.AluOpType.mult
ADD = mybir.AluOpType.add
MIN = mybir.AluOpType.min
LAG = 5
import os
# tensor_tensor_reduce crashes/hangs on HW in this usage (in0=PSUM,
# scale=-1, op1=min) — keep the 2-op fallback. scalar_tensor_tensor works.
USE_TTR = os.environ.get("K_TTR", "0") == "1"
USE_STT = os.environ.get("K_STT", "1") == "1"


class _Ctx:
    pass


def _body(tc, out_ext, ctx_in, q_in, wq_in, wc_in, wm_in):
    nc = tc.nc
    with (
        tc.tile_pool(name="singles", bufs=1) as singles,
        tc.tile_pool(name="stgp", bufs=BPC * NG) as stgp,
        tc.tile_pool(name="xc16p", bufs=BPC * NG) as xc16p,
        tc.tile_pool(name="qside", bufs=2) as qside,
        tc.tile_pool(name="work", bufs=4) as work,
        tc.tile_pool(name="pers", bufs=2 * NP) as pers,
        tc.tile_pool(name="statsp", bufs=2) as statsp,
        tc.tile_pool(name="ps_xct", bufs=2, space="PSUM") as ps_xct,
        tc.tile_pool(name="ps_sim", bufs=2, space="PSUM") as ps_sim,
        tc.tile_pool(name="ps_pt", bufs=1, space="PSUM") as ps_pt,
        tc.tile_pool(name="ps_c2q", bufs=1, space="PSUM") as ps_c2q,
        tc.tile_pool(name="ps_misc", bufs=1, space="PSUM") as ps_misc,
    ):
        # ---- constants + params ------------------------------------------
        ident = singles.tile([128, 128], F32)
        make_identity(nc, ident)
        ident_h = singles.tile([128, 128], F16)
        make_identity(nc, ident_h)
        ones_r = singles.tile([1, 128], F32)
        nc.vector.memset(ones_r, 1.0)
        ones_c = singles.tile([128, 1], F32)
        nc.vector.memset(ones_c, 1.0)
        wq_sb = singles.tile([128, 2], F32)
        nc.sync.dma_start(out=wq_sb, in_=wq_in.rearrange("(j p) -> p j", p=128))
        wc_sb = singles.tile([128, 2], F32)
        nc.sync.dma_start(out=wc_sb, in_=wc_in.rearrange("(j p) -> p j", p=128))
        wm_sb = singles.tile([128, 2], F32)
        nc.sync.dma_start(out=wm_sb, in_=wm_in.rearrange("(j p) -> p j", p=128))

        # ---- all loads up front + copy-through of cols 0:E ---------------
        bs = []
        for b in range(BPC):
            st = _Ctx()
            bs.append(st)
            st.qm = qside.tile([128, E], F32, tag="qm", name="qm")
            nc.sync.dma_start(out=st.qm, in_=q_in[b])
        for b in range(BPC):
            st = bs[b]
            st.stgs = []
            for g in range(NG):
                stg = stgp.tile([128, 4, 4 * E], F32, tag="stg", name="stg")
                st.stgs.append(stg)
                nc.sync.dma_start(
                    out=stg[:, :, 0:E],
                    in_=ctx_in[b, g * 512 : (g + 1) * 512, :].rearrange(
                        "(t p) e -> p t e", p=128
                    ),
                )
        def copythru(b, g):
            # out[:, :, 0:E] is exactly the context; issued on the gpsimd
            # SWDGE queue so it drains independently of the sync queue --
            # always-ready filler traffic for any store-readiness stall.
            nc.gpsimd.dma_start(
                out=out_ext[b, g * 512 : (g + 1) * 512, 0:E].rearrange(
                    "(t p) f -> p t f", p=128
                ),
                in_=bs[b].stgs[g][:, :, 0:E],
            )

        # ---- question-side prep for both batches -------------------------
        for b in range(BPC):
            st = bs[b]
            qm = st.qm
            qmt_ps = ps_xct.tile([128, E], F32, tag="xct", name="qmt_ps")
            for j in range(2):
                nc.tensor.transpose(
                    qmt_ps[:, j * 128 : (j + 1) * 128],
                    qm[:, j * 128 : (j + 1) * 128],
                    ident,
                )
            qmt_sb = qside.tile([128, E], F32, tag="qmt", name="qmt_sb")
            nc.vector.tensor_copy(out=qmt_sb, in_=qmt_ps)
            # fp16 question for the c2q matmul
            st.qm16 = qside.tile([128, E], F16, tag="qm16", name="qm16")
            nc.vector.tensor_copy(out=st.qm16, in_=qm)
            # rhs_aug[:, j, 0:128] = wm-chunk * QmT-chunk ; [:, j, 128] = wc
            # col 129 is zero pad to keep the PE output width even (N=130).
            st.rhs_aug = qside.tile([128, 2, 130], F16, tag="rhs_aug",
                                    name="rhs_aug")
            for j in range(2):
                nc.vector.tensor_scalar_mul(
                    st.rhs_aug[:, j, 0:128],
                    qmt_sb[:, j * 128 : (j + 1) * 128],
                    wm_sb[:, j : j + 1],
                )
                nc.vector.tensor_copy(
                    out=st.rhs_aug[:, j, 128:129], in_=wc_sb[:, j : j + 1]
                )
                nc.vector.memset(st.rhs_aug[:, j, 129:130], 0.0)
            qw_ps = ps_misc.tile([1, 128], F32, tag="misc", name="qw_ps")
            for j in range(2):
                nc.tensor.matmul(
                    qw_ps,
                    wq_sb[:, j : j + 1],
                    qmt_sb[:, j * 128 : (j + 1) * 128],
                    start=(j == 0),
                    stop=(j == 1),
                )
            qw_row = qside.tile([1, 128], F32, tag="qw_row", name="qw_row")
            nc.vector.tensor_copy(out=qw_row, in_=qw_ps)
            qwb_ps = ps_misc.tile([128, 128], F32, tag="misc", name="qwb_ps")
            nc.tensor.matmul(qwb_ps, ones_r, qw_row, start=True, stop=True)
            st.qwb2 = qside.tile([128, 2, 128], F32, tag="qwb2", name="qwb2")
            nc.vector.tensor_copy(out=st.qwb2[:, 0, :], in_=qwb_ps)
            nc.vector.tensor_copy(out=st.qwb2[:, 1, :], in_=qwb_ps)
            st.mstat = statsp.tile([128, NT], F32, tag="mstat", name="mstat")
            st.s_all = statsp.tile([128, NP, 2], F32, tag="s_all", name="s_all")
            st.r_all = statsp.tile([128, NP, 2], F32, tag="r_all", name="r_all")
            st.p_sbs = {}
            st.xc16s = []

        # ---- pass A: sim + softmax stats + exp ---------------------------
        def pass_a(b, k):
            st = bs[b]
            g, h = k // 2, k % 2
            stg = st.stgs[g]
            if h == 0:
                xc16 = xc16p.tile([128, 4, E], F16, tag="xc16", name="xc16")
                nc.scalar.copy(out=xc16, in_=stg[:, :, 0:E])
                st.xc16s.append(xc16)
            xc16 = st.xc16s[g]
            xct_ps = ps_xct.tile([128, 2, E], F16, tag="xct", name="xct_ps")
            for i in range(2):
                for j in range(2):
                    nc.tensor.transpose(
                        xct_ps[:, i, j * 128 : (j + 1) * 128],
                        xc16[:, 2 * h + i, j * 128 : (j + 1) * 128],
                        ident_h,
                    )
            xct_sb = work.tile([128, 2, E], F16, tag="xct_sb", name="xct_sb")
            nc.vector.tensor_copy(out=xct_sb, in_=xct_ps)
            sim_ps = ps_sim.tile([128, 2, 130], F32, tag="sim", name="sim_ps")
            for i in range(2):
                for j in range(2):
                    nc.tensor.matmul(
                        sim_ps[:, i, :],
                        xct_sb[:, i, j * 128 : (j + 1) * 128],
                        st.rhs_aug[:, j, :],
                        start=(j == 0),
                        stop=(j == 1),
                    )
            # nsim = -(sim + qw); nm = min(nsim) = -rowmax  (fused on DVE)
            nsim = work.tile([128, 2, 128], F32, tag="nsim", name="nsim")
            nm = work.tile([128, 2], F32, tag="nm", name="nm")
            if USE_TTR:
                for i in range(2):
                    nc.vector.tensor_tensor_reduce(
                        out=nsim[:, i, :],
                        in0=sim_ps[:, i, 0:128],
                        in1=st.qwb2[:, i, :],
                        scale=-1.0,
                        scalar=3.0e38,
                        op0=ADD,
                        op1=MIN,
                        accum_out=nm[:, i : i + 1],
                    )
            else:
                # fallback: sim_in kept positive, nm = -rowmax via negate;
                # the exp then runs with scale=+1 (v1 style).
                nc.vector.tensor_add(nsim, sim_ps[:, :, 0:128], st.qwb2)
                nc.vector.reduce_max(out=nm, in_=nsim, axis=AX, negate=True)
            # mstat = cwc + rowmax = cwc - nm
            nc.vector.tensor_sub(
                st.mstat[:, 2 * k : 2 * k + 2], sim_ps[:, :, 128], nm
            )
            p_sb = pers.tile([128, 2, 128], F16, tag="p_sb", name="p_sb")
            for i in range(2):
                # p = exp(-nsim + nm) = exp(sim + qw - rowmax); row-sum freed
                # into s_all by the activation accumulator.
                nc.scalar.activation(
                    out=p_sb[:, i, :],
                    in_=nsim[:, i, :],
                    func=EXP,
                    bias=nm[:, i : i + 1],
                    scale=-1.0 if USE_TTR else 1.0,
                    accum_out=st.s_all[:, k, i : i + 1],
                )
            st.p_sbs[k] = p_sb

        # ---- pass B: c2q + ctx*c2q + store cols E:3E ---------------------
        def pass_b(b, k):
            st = bs[b]
            g, h = k // 2, k % 2
            stg = st.stgs[g]
            xc16 = st.xc16s[g]
            p_sb = st.p_sbs[k]
            pt_ps = st.pt_ring[:, k % 4, :, :]
            for i in range(2):
                nc.tensor.transpose(pt_ps[:, i, :], p_sb[:, i, :], ident_h)
            pt_sb = work.tile([128, 2, 128], F16, tag="pt_sb", name="pt_sb")
            nc.vector.tensor_copy(out=pt_sb, in_=pt_ps)
            nc.vector.reciprocal(
                out=st.r_all[:, k, :], in_=st.s_all[:, k, :]
            )
            for i in range(2):
                c2q_ps = st.c2q_ring[:, k % 2, i, :]
                nc.tensor.matmul(
                    c2q_ps, pt_sb[:, i, :], st.qm16, start=True, stop=True
                )
                nc.scalar.activation(
                    out=stg[:, 2 * h + i, E : 2 * E],
                    in_=c2q_ps,
                    func=CPY,
                    scale=st.r_all[:, k, i : i + 1],
                )
                if k % 2 == 0 and USE_STT:
                    # ctx*c2q = (c2q_raw * 1/s) * ctx, straight out of PSUM
                    # (DVE only: gpsimd has no PSUM access)
                    nc.vector.scalar_tensor_tensor(
                        out=stg[:, 2 * h + i, 2 * E : 3 * E],
                        in0=c2q_ps,
                        scalar=st.r_all[:, k, i : i + 1],
                        in1=xc16[:, 2 * h + i, :],
                        op0=MUL,
                        op1=MUL,
                    )
                elif k % 2 == 0:
                    nc.vector.tensor_mul(
                        stg[:, 2 * h + i, 2 * E : 3 * E],
                        stg[:, 2 * h + i, E : 2 * E],
                        xc16[:, 2 * h + i, :],
                    )
            if k % 2 == 1:
                # odd pairs: SBUF-side product on gpsimd off the rescaled c2q
                nc.gpsimd.tensor_mul(
                    stg[:, 2 * h : 2 * h + 2, 2 * E : 3 * E],
                    stg[:, 2 * h : 2 * h + 2, E : 2 * E],
                    xc16[:, 2 * h : 2 * h + 2, :],
                )
            r0 = g * 512 + h * 256
            nc.sync.dma_start(
                out=out_ext[b, r0 : r0 + 256, E : 3 * E].rearrange(
                    "(t p) f -> p t f", p=128
                ),
                in_=stg[:, 2 * h : 2 * h + 2, E : 3 * E],
            )

        # ---- q2c epilogue: softmax over C, broadcast weights -------------
        def ep_pre(b):
            st = bs[b]
            mstat = st.mstat
            r1 = statsp.tile([128, 1], F32, tag="r1", name="r1")
            nc.vector.reduce_max(out=r1, in_=mstat, axis=AX)
            r1t_ps = ps_misc.tile([1, 128], F32, tag="misc", name="r1t_ps")
            nc.tensor.transpose(r1t_ps, r1, ident)
            neg_gmax = statsp.tile([1, 1], F32, tag="gmax", name="neg_gmax")
            nc.vector.reduce_max(
                out=neg_gmax, in_=r1t_ps, axis=AX, negate=True
            )
            ngb_ps = ps_misc.tile([128, 1], F32, tag="misc", name="ngb_ps")
            nc.tensor.matmul(ngb_ps, ones_r, neg_gmax, start=True, stop=True)
            ngb_sb = statsp.tile([128, 1], F32, tag="ngb", name="ngb_sb")
            nc.vector.tensor_copy(out=ngb_sb, in_=ngb_ps)
            st.e_sb = statsp.tile([128, NT], F16, tag="e_sb", name="e_sb")
            s_col = statsp.tile([128, 1], F32, tag="s_col", name="s_col")
            nc.scalar.activation(
                out=st.e_sb, in_=mstat, func=EXP, bias=ngb_sb, scale=1.0,
                accum_out=s_col,
            )
            tot_ps = ps_misc.tile([1, 1], F32, tag="misc", name="tot_ps")
            nc.tensor.matmul(tot_ps, s_col, ones_c, start=True, stop=True)
            st.rt_sb = statsp.tile([1, 1], F32, tag="rt", name="rt_sb")
            nc.vector.reciprocal(out=st.rt_sb, in_=tot_ps)

        def ep_q2c(b, half):
            st = bs[b]
            if half == 0:
                st.q2c_ps = ps_misc.tile([1, E], F32, tag="misc",
                                         name="q2c_ps")
            for t in range(half * NT // 2, (half + 1) * NT // 2):
                nc.tensor.matmul(
                    st.q2c_ps,
                    st.e_sb[:, t : t + 1],
                    st.xc16s[t // 4][:, t % 4, :],
                    start=(t == 0),
                    stop=(t == NT - 1),
                )

        def ep_fin(b):
            st = bs[b]
            q2c_sb = statsp.tile([1, E], F32, tag="q2c_sb", name="q2c_sb")
            nc.scalar.activation(
                out=q2c_sb, in_=st.q2c_ps, func=CPY, scale=st.rt_sb
            )
            q2cb_ps = ps_misc.tile([128, E], F32, tag="misc", name="q2cb_ps")
            nc.tensor.matmul(q2cb_ps, ones_r, q2c_sb, start=True, stop=True)
            st.q2cb16 = statsp.tile([128, 2, E], F16, tag="q2cb", name="q2cb16")
            nc.vector.tensor_copy(out=st.q2cb16[:, 0, :], in_=q2cb_ps)
            nc.vector.tensor_copy(out=st.q2cb16[:, 1, :], in_=q2cb_ps)

        # ---- ctx * q2c + store cols 3E:4E --------------------------------
        def stage3(b, g):
            st = bs[b]
            stg = st.stgs[g]
            xc16 = st.xc16s[g]
            for h in range(2):
                # mid-kernel (batch 0) stage3 runs fully on gpsimd (DVE is
                # the loaded engine there); the tail (batch 1) splits V/G.
                eng = nc.vector if (b == 1 and h == 0) else nc.gpsimd
                eng.tensor_mul(
                    stg[:, 2 * h : 2 * h + 2, 3 * E : 4 * E],
                    xc16[:, 2 * h : 2 * h + 2, :],
                    st.q2cb16,
                )
            nc.gpsimd.dma_start(
                out=out_ext[
                    b, g * 512 : (g + 1) * 512, 3 * E : 4 * E
                ].rearrange("(t p) f -> p t f", p=128),
                in_=stg[:, :, 3 * E : 4 * E],
            )

        # ---- schedule ----------------------------------------------------
        for b in range(BPC):
            bs[b].pt_ring = ps_pt.tile(
                [128, 4, 2, 128], F16, tag="pt", name="pt_ring"
            )
            bs[b].c2q_ring = ps_c2q.tile(
                [128, 2, 2, E], F32, tag="c2q", name="c2q_ring"
            )
        # Modulo schedule over global pair index kk = b*NP + k.  Pass B lags
        # pass A by LAG pairs; pass B is emitted first inside each round
        # (its inputs are oldest, hence ready).  The q2c epilogue chain for
        # each batch starts right after that batch's pass A drains and is
        # split into small pieces so it never parks mid-queue in front of
        # ready pass-B work.
        TOT = BPC * NP
        for r in range(TOT + LAG + NG + 1):
            if r >= LAG and r - LAG < TOT:
                kk = r - LAG
                pass_b(kk // NP, kk % NP)
            if r < TOT:
                pass_a(r // NP, r % NP)
            if 1 <= r <= BPC * NG:
                g = r - 1
                copythru(g // NG, g % NG)
            if r == NP:
                ep_pre(0)
            elif r == NP + 1:
                ep_q2c(0, 0)
            elif r == NP + 2:
                ep_q2c(0, 1)
            elif r == NP + 3:
                ep_fin(0)
            elif NP + 4 <= r < NP + 4 + 2 * NG and (r - NP) % 2 == 0:
                stage3(0, (r - (NP + 4)) // 2)
            if r == TOT:
                ep_pre(1)
            elif r == TOT + 1:
                ep_q2c(1, 0)
                ep_q2c(1, 1)
            elif r == TOT + 2:
                ep_fin(1)
            elif TOT + 3 <= r < TOT + 3 + NG:
                stage3(1, r - (TOT + 3))


_NC_CACHE = None


def _build():
    global _NC_CACHE
    if _NC_CACHE is not None:
        return _NC_CACHE
    nc = bacc.Bacc(
        "TRN2", target_bir_lowering=False, debug=False, num_devices=NCORES
    )
    ctx_in = nc.dram_tensor("context", [BPC, C, E], F32, kind="ExternalInput").ap()
    q_in = nc.dram_tensor("question", [BPC, Q, E], F32, kind="ExternalInput").ap()
    wq_in = nc.dram_tensor("w_question", [E], F32, kind="ExternalInput").ap()
    wc_in = nc.dram_tensor("w_context", [E], F32, kind="ExternalInput").ap()
    wm_in = nc.dram_tensor("w_multiple", [E], F32, kind="ExternalInput").ap()
    out_ext = nc.dram_tensor("out", [BPC, C, 4 * E], F32, kind="ExternalOutput").ap()
    with tile.TileContext(nc) as tc:
        _body(tc, out_ext, ctx_in, q_in, wq_in, wc_in, wm_in)
    nc.compile()
    _NC_CACHE = nc
    return nc


def _run(inputs, trace=False, **kw):
    nc = _build()
    context = np.ascontiguousarray(np.asarray(inputs["context"], dtype=np.float32))
    question = np.ascontiguousarray(np.asarray(inputs["question"], dtype=np.float32))
    wq = np.ascontiguousarray(np.asarray(inputs["w_question"], dtype=np.float32))
    wc = np.ascontiguousarray(np.asarray(inputs["w_context"], dtype=np.float32))
    wm = np.ascontiguousarray(np.asarray(inputs["w_multiple"], dtype=np.float32))
    in_maps = []
    for i in range(NCORES):
        sl = slice(i * BPC, (i + 1) * BPC)
        in_maps.append(
            {
                "context": context[sl],
                "question": question[sl],
                "w_question": wq,
                "w_context": wc,
                "w_multiple": wm,
            }
        )
    res = run_bass_kernel_spmd(
        nc, in_maps, core_ids=list(range(NCORES)), trace=trace, **kw
    )
    out = np.concatenate([res.results[i]["out"] for i in range(NCORES)], axis=0)
    return out, res


def kernel(**inputs):
    try:
        out, _ = _run(inputs, trace=False)
    except Exception:
        # transient device errors (e.g. a wedged core from a prior run)
        # usually clear on retry
        out, _ = _run(inputs, trace=False)
    return out


# revision 22
# speedup vs baseline: 1.3001x; 1.0556x over previous
"""Trainium2 Bass kernel for BiDAF-style bidirectional attention.

Reference math (per batch b):
    sim[c,q]  = q[q]·wq + c[c]·wc + sum_e wm[e]*question[q,e]*context[c,e]
    c2q[c,:]  = softmax_q(sim[c,:]) @ question          # (C, E)
    q2c[:]    = softmax_c(max_q sim[c,:]) @ context     # (E,)
    out[c,:]  = [context | c2q | context*c2q | context*q2c]

Sharding: pure data parallel over batch (B=16 -> 2 batches per core x 8 cores).

Two-pass pipeline, one batch phase-shifted against the other:
  - all DMA loads are emitted up front; output cols 0:E are a verbatim
    copy of the context, so "copy-through" stores stream them out right
    after each group load lands -- DMA is busy from the first microsecond.
  - pass A (per pair of context tiles): PE transpose -> f32r sim matmul
    (wc folded in as an extra output column) -> qw add + rowmax -> exp
    (bf16 attention weights).
  - pass B (per pair): attention-weight transpose (4-deep ring packed in
    one PSUM bank) -> bf16 c2q matmul whose 257th column is the softmax
    row-sum (ones column appended to the question, padded to N=258) ->
    reciprocal + row rescale -> ctx*c2q on gpsimd -> store cols E:3E.
  - emission is a modulo software pipeline over the global pair index:
    pass B lags pass A by 5 pairs and is emitted first within each round,
    so every engine's in-order queue meets its operands already ready.
    The serial q2c epilogue chain is split into small pieces (pre / two
    matmul halves / fin) spread across rounds so it never parks mid-queue
    in front of ready pass-B work; ctx*q2c stores ride the scalar-engine
    HWDGE ring to dodge sync-ring head-of-line blocking.
  - q2c rank-1 matmuls use a bf16 shadow of the context cast during
    pass A; the tail is pure DMA (stores of the last batch's columns).
"""

import numpy as np

import concourse.bass as bass
import concourse.tile as tile
import concourse.mybir as mybir
from concourse import bacc
from concourse.bass_utils import run_bass_kernel_spmd
from concourse.masks import make_identity

B, C, Q, E = 16, 2048, 128, 256
NCORES = 8
BPC = B // NCORES          # batches per core
NT = C // 128              # context tiles per batch
NG = NT // 4               # groups of 4 tiles
NP = NT // 2               # pairs per batch
F32 = mybir.dt.float32
F32R = mybir.dt.float32r
BF16 = mybir.dt.bfloat16
AX = mybir.AxisListType.X
EXP = mybir.ActivationFunctionType.Exp
CPY = mybir.ActivationFunctionType.Copy


class _Ctx:
    pass


def _body(tc, out_ext, ctx_in, q_in, wq_in, wc_in, wm_in):
    nc = tc.nc
    with (
        tc.tile_pool(name="singles", bufs=1) as singles,
        tc.tile_pool(name="stgp", bufs=BPC * NG) as stgp,
        tc.tile_pool(name="qside", bufs=2) as qside,
        tc.tile_pool(name="xbfp", bufs=BPC * NG) as xbfp,
        tc.tile_pool(name="work", bufs=6) as work,
        tc.tile_pool(name="pers", bufs=2 * NP) as pers,
        tc.tile_pool(name="statsp", bufs=2) as statsp,
        tc.tile_pool(name="ps_xct", bufs=2, space="PSUM") as ps_xct,
        tc.tile_pool(name="ps_sim", bufs=2, space="PSUM") as ps_sim,
        tc.tile_pool(name="ps_pt", bufs=1, space="PSUM") as ps_pt,
        tc.tile_pool(name="ps_c2q", bufs=2, space="PSUM") as ps_c2q,
        tc.tile_pool(name="ps_misc", bufs=1, space="PSUM") as ps_misc,
    ):
        # ---- constants + params ------------------------------------------
        ident = singles.tile([128, 128], F32)
        make_identity(nc, ident)
        ident_bf = singles.tile([128, 128], BF16)
        make_identity(nc, ident_bf)
        ones_r = singles.tile([1, 128], F32)
        nc.vector.memset(ones_r, 1.0)
        ones_c = singles.tile([128, 1], F32)
        nc.vector.memset(ones_c, 1.0)
        wq_sb = singles.tile([128, 2], F32)
        nc.sync.dma_start(out=wq_sb, in_=wq_in.rearrange("(j p) -> p j", p=128))
        wc_sb = singles.tile([128, 2], F32)
        nc.sync.dma_start(out=wc_sb, in_=wc_in.rearrange("(j p) -> p j", p=128))
        wm_sb = singles.tile([128, 2], F32)
        nc.sync.dma_start(out=wm_sb, in_=wm_in.rearrange("(j p) -> p j", p=128))

        # ---- all loads up front + copy-through of cols 0:E ---------------
        bs = []
        for b in range(BPC):
            st = _Ctx()
            bs.append(st)
            st.qm = qside.tile([128, E], F32, tag="qm", name="qm")
            nc.sync.dma_start(out=st.qm, in_=q_in[b])
        for b in range(BPC):
            st = bs[b]
            st.stgs = []
            for g in range(NG):
                stg = stgp.tile([128, 4, 4 * E], F32, tag="stg", name="stg")
                st.stgs.append(stg)
                nc.sync.dma_start(
                    out=stg[:, :, 0:E],
                    in_=ctx_in[b, g * 512 : (g + 1) * 512, :].rearrange(
                        "(t p) e -> p t e", p=128
                    ),
                )
                # out[:, :, 0:E] is exactly the context: stream it out now
                nc.sync.dma_start(
                    out=out_ext[b, g * 512 : (g + 1) * 512, 0:E].rearrange(
                        "(t p) f -> p t f", p=128
                    ),
                    in_=stg[:, :, 0:E],
                )

        # ---- question-side prep for both batches -------------------------
        for b in range(BPC):
            st = bs[b]
            qm = st.qm
            qmt_ps = ps_xct.tile([128, E], F32, tag="xct", name="qmt_ps")
            for j in range(2):
                nc.tensor.transpose(
                    qmt_ps[:, j * 128 : (j + 1) * 128],
                    qm[:, j * 128 : (j + 1) * 128],
                    ident,
                )
            qmt_sb = qside.tile([128, E], F32, tag="qmt", name="qmt_sb")
            nc.vector.tensor_copy(out=qmt_sb, in_=qmt_ps)
            # question in bf16 with a ones column appended: the c2q matmul
            # then emits the softmax row-sum as its 257th output column
            # (padded to N=258 to keep the PE output width even).
            st.qm_bf = qside.tile([128, E + 2], BF16, tag="qmbf", name="qm_bf")
            nc.vector.tensor_copy(out=st.qm_bf[:, 0:E], in_=qm)
            nc.vector.memset(st.qm_bf[:, E : E + 1], 1.0)
            nc.vector.memset(st.qm_bf[:, E + 1 : E + 2], 0.0)
            # rhs_aug[:, j, 0:128] = wm-chunk * QmT-chunk ; [:, j, 128] = wc
            # cols 129:256 are zero pad so the fp32r matmul runs at N=256.
            st.rhs_aug = qside.tile([128, 2, E], F32R, tag="rhs_aug",
                                    name="rhs_aug")
            for j in range(2):
                nc.vector.tensor_scalar_mul(
                    st.rhs_aug[:, j, 0:128],
                    qmt_sb[:, j * 128 : (j + 1) * 128],
                    wm_sb[:, j : j + 1],
                )
                nc.vector.tensor_copy(
                    out=st.rhs_aug[:, j, 128:129], in_=wc_sb[:, j : j + 1]
                )
                nc.vector.tensor_scalar_mul(
                    st.rhs_aug[:, j, 129:256],
                    qmt_sb[:, j * 128 : (j + 1) * 128][:, 0:127],
                    0.0,
                )
            qw_ps = ps_misc.tile([1, 128], F32, tag="misc", name="qw_ps")
            for j in range(2):
                nc.tensor.matmul(
                    qw_ps,
                    wq_sb[:, j : j + 1],
                    qmt_sb[:, j * 128 : (j + 1) * 128],
                    start=(j == 0),
                    stop=(j == 1),
                )
            qw_row = qside.tile([1, 128], F32, tag="qw_row", name="qw_row")
            nc.vector.tensor_copy(out=qw_row, in_=qw_ps)
            qwb_ps = ps_misc.tile([128, 128], F32, tag="misc", name="qwb_ps")
            nc.tensor.matmul(qwb_ps, ones_r, qw_row, start=True, stop=True)
            st.qwb2 = qside.tile([128, 2, 128], F32, tag="qwb2", name="qwb2")
            nc.vector.tensor_copy(out=st.qwb2[:, 0, :], in_=qwb_ps)
            nc.vector.tensor_copy(out=st.qwb2[:, 1, :], in_=qwb_ps)
            st.mstat = statsp.tile([128, NT], F32, tag="mstat", name="mstat")
            st.p_sbs = {}
            st.recips = {}
            st.xcbfs = []

        # ---- pass A: sim + softmax stats + exp ---------------------------
        def pass_a(b, k):
            st = bs[b]
            g, h = k // 2, k % 2
            stg = st.stgs[g]
            if h == 0:
                xcbf = xbfp.tile([128, 4, E], BF16, tag="xcbf", name="xcbf")
                nc.scalar.copy(out=xcbf, in_=stg[:, :, 0:E])
                st.xcbfs.append(xcbf)
            xct_ps = ps_xct.tile([128, 2, E], F32, tag="xct", name="xct_ps")
            for i in range(2):
                for j in range(2):
                    nc.tensor.transpose(
                        xct_ps[:, i, j * 128 : (j + 1) * 128],
                        stg[:, 2 * h + i, j * 128 : (j + 1) * 128],
                        ident,
                    )
            xct_sb = work.tile([128, 2, E], F32R, tag="xct_sb", name="xct_sb")
            nc.vector.tensor_copy(out=xct_sb, in_=xct_ps)
            sim_ps = ps_sim.tile([128, 2, E], F32, tag="sim", name="sim_ps")
            for i in range(2):
                for j in range(2):
                    nc.tensor.matmul(
                        sim_ps[:, i, :],
                        xct_sb[:, i, j * 128 : (j + 1) * 128],
                        st.rhs_aug[:, j, :],
                        start=(j == 0),
                        stop=(j == 1),
                    )
            sim_in = work.tile([128, 2, 128], F32, tag="sim_in", name="sim_in")
            nc.vector.tensor_add(sim_in, sim_ps[:, :, 0:128], st.qwb2)
            neg_m = work.tile([128, 2], F32, tag="neg_m", name="neg_m")
            nc.vector.reduce_max(out=neg_m, in_=sim_in, axis=AX, negate=True)
            nc.vector.tensor_sub(
                st.mstat[:, 2 * k : 2 * k + 2], sim_ps[:, :, 128], neg_m
            )
            p_sb = pers.tile([128, 2, 128], BF16, tag="p_sb", name="p_sb")
            for i in range(2):
                nc.scalar.activation(
                    out=p_sb[:, i, :],
                    in_=sim_in[:, i, :],
                    func=EXP,
                    bias=neg_m[:, i : i + 1],
                    scale=1.0,
                )
            st.p_sbs[k] = p_sb

        # ---- pass B: c2q + ctx*c2q + store cols E:3E ---------------------
        def pass_b(b, k):
            st = bs[b]
            g, h = k // 2, k % 2
            stg = st.stgs[g]
            p_sb = st.p_sbs[k]
            pt_ps = st.pt_ring[:, k % 4, :, :]
            for i in range(2):
                nc.tensor.transpose(pt_ps[:, i, :], p_sb[:, i, :], ident_bf)
            pt_sb = work.tile([128, 2, 128], BF16, tag="pt_sb", name="pt_sb")
            nc.vector.tensor_copy(out=pt_sb, in_=pt_ps)
            recip = work.tile([128, 2], F32, tag="recip", name="recip")
            for i in range(2):
                c2q_ps = ps_c2q.tile(
                    [128, E + 2], F32, tag="c2q", name="c2q_ps"
                )
                nc.tensor.matmul(
                    c2q_ps, pt_sb[:, i, :], st.qm_bf, start=True, stop=True
                )
                nc.vector.reciprocal(
                    out=recip[:, i : i + 1], in_=c2q_ps[:, E : E + 1]
                )
                nc.scalar.activation(
                    out=stg[:, 2 * h + i, E : 2 * E],
                    in_=c2q_ps[:, 0:E],
                    func=CPY,
                    scale=recip[:, i : i + 1],
                )
            nc.gpsimd.tensor_mul(
                stg[:, 2 * h : 2 * h + 2, 2 * E : 3 * E],
                stg[:, 2 * h : 2 * h + 2, 0:E],
                stg[:, 2 * h : 2 * h + 2, E : 2 * E],
            )
            r0 = g * 512 + h * 256
            nc.sync.dma_start(
                out=out_ext[b, r0 : r0 + 256, E : 3 * E].rearrange(
                    "(t p) f -> p t f", p=128
                ),
                in_=stg[:, 2 * h : 2 * h + 2, E : 3 * E],
            )

        # ---- q2c epilogue: softmax over C, broadcast weights -------------
        def ep_pre(b):
            st = bs[b]
            mstat = st.mstat
            r1 = statsp.tile([128, 1], F32, tag="r1", name="r1")
            nc.vector.reduce_max(out=r1, in_=mstat, axis=AX)
            r1t_ps = ps_misc.tile([1, 128], F32, tag="misc", name="r1t_ps")
            nc.tensor.transpose(r1t_ps, r1, ident)
            neg_gmax = statsp.tile([1, 1], F32, tag="gmax", name="neg_gmax")
            nc.vector.reduce_max(
                out=neg_gmax, in_=r1t_ps, axis=AX, negate=True
            )
            ngb_ps = ps_misc.tile([128, 1], F32, tag="misc", name="ngb_ps")
            nc.tensor.matmul(ngb_ps, ones_r, neg_gmax, start=True, stop=True)
            ngb_sb = statsp.tile([128, 1], F32, tag="ngb", name="ngb_sb")
            nc.vector.tensor_copy(out=ngb_sb, in_=ngb_ps)
            st.e_sb = statsp.tile([128, NT], BF16, tag="e_sb", name="e_sb")
            s_col = statsp.tile([128, 1], F32, tag="s_col", name="s_col")
            nc.scalar.activation(
                out=st.e_sb, in_=mstat, func=EXP, bias=ngb_sb, scale=1.0,
                accum_out=s_col,
            )
            tot_ps = ps_misc.tile([1, 1], F32, tag="misc", name="tot_ps")
            nc.tensor.matmul(tot_ps, s_col, ones_c, start=True, stop=True)
            st.rt_sb = statsp.tile([1, 1], F32, tag="rt", name="rt_sb")
            nc.vector.reciprocal(out=st.rt_sb, in_=tot_ps)

        def ep_q2c(b, half):
            st = bs[b]
            if half == 0:
                st.q2c_ps = ps_misc.tile([1, E], F32, tag="misc",
                                         name="q2c_ps")
            for t in range(half * NT // 2, (half + 1) * NT // 2):
                nc.tensor.matmul(
                    st.q2c_ps,
                    st.e_sb[:, t : t + 1],
                    st.xcbfs[t // 4][:, t % 4, :],
                    start=(t == 0),
                    stop=(t == NT - 1),
                )

        def ep_fin(b):
            st = bs[b]
            q2c_sb = statsp.tile([1, E], F32, tag="q2c_sb", name="q2c_sb")
            nc.scalar.activation(
                out=q2c_sb, in_=st.q2c_ps, func=CPY, scale=st.rt_sb
            )
            q2cb_ps = ps_misc.tile([128, E], F32, tag="misc", name="q2cb_ps")
            nc.tensor.matmul(q2cb_ps, ones_r, q2c_sb, start=True, stop=True)
            st.q2cb2 = statsp.tile([128, 2, E], F32, tag="q2cb", name="q2cb2")
            nc.vector.tensor_copy(out=st.q2cb2[:, 0, :], in_=q2cb_ps)
            nc.vector.tensor_copy(out=st.q2cb2[:, 1, :], in_=q2cb_ps)

        # ---- ctx * q2c + store cols 3E:4E --------------------------------
        def stage3(b, g):
            st = bs[b]
            stg = st.stgs[g]
            for h in range(2):
                eng = nc.vector if h == 0 else nc.gpsimd
                eng.tensor_mul(
                    stg[:, 2 * h : 2 * h + 2, 3 * E : 4 * E],
                    stg[:, 2 * h : 2 * h + 2, 0:E],
                    st.q2cb2,
                )
            nc.scalar.dma_start(
                out=out_ext[
                    b, g * 512 : (g + 1) * 512, 3 * E : 4 * E
                ].rearrange("(t p) f -> p t f", p=128),
                in_=stg[:, :, 3 * E : 4 * E],
            )

        # ---- schedule ----------------------------------------------------
        for b in range(BPC):
            bs[b].pt_ring = ps_pt.tile(
                [128, 4, 2, 128], BF16, tag="pt", name="pt_ring"
            )
        # Modulo schedule over global pair index kk = b*NP + k.  Pass B lags
        # pass A by LAG pairs; pass B is emitted first inside each round
        # (its inputs are oldest, hence ready).  The serial q2c epilogue
        # chain is split into small pieces spread across rounds so it never
        # parks mid-queue in front of ready pass-B work.
        LAG = 5
        TOT = BPC * NP
        for r in range(TOT + LAG + NG):
            if r >= LAG and r - LAG < TOT:
                kk = r - LAG
                pass_b(kk // NP, kk % NP)
            if r < TOT:
                pass_a(r // NP, r % NP)
            if r == NP:
                ep_pre(0)
            elif r == NP + 1:
                ep_q2c(0, 0)
            elif r == NP + 2:
                ep_q2c(0, 1)
            elif r == NP + 3:
                ep_fin(0)
            elif NP + 4 <= r < NP + 4 + 2 * NG and (r - NP) % 2 == 0:
                stage3(0, (r - (NP + 4)) // 2)
            if r == TOT:
                ep_pre(1)
            elif r == TOT + 1:
                ep_q2c(1, 0)
                ep_q2c(1, 1)
            elif r == TOT + 2:
                ep_fin(1)
            elif TOT + 3 <= r < TOT + 3 + NG:
                stage3(1, r - (TOT + 3))


_NC_CACHE = None


def _build():
    global _NC_CACHE
    if _NC_CACHE is not None:
        return _NC_CACHE
    nc = bacc.Bacc(
        "TRN2", target_bir_lowering=False, debug=False, num_devices=NCORES
    )
    ctx_in = nc.dram_tensor("context", [BPC, C, E], F32, kind="ExternalInput").ap()
    q_in = nc.dram_tensor("question", [BPC, Q, E], F32, kind="ExternalInput").ap()
    wq_in = nc.dram_tensor("w_question", [E], F32, kind="ExternalInput").ap()
    wc_in = nc.dram_tensor("w_context", [E], F32, kind="ExternalInput").ap()
    wm_in = nc.dram_tensor("w_multiple", [E], F32, kind="ExternalInput").ap()
    out_ext = nc.dram_tensor("out", [BPC, C, 4 * E], F32, kind="ExternalOutput").ap()
    with tile.TileContext(nc) as tc:
        _body(tc, out_ext, ctx_in, q_in, wq_in, wc_in, wm_in)
    nc.compile()
    _NC_CACHE = nc
    return nc


def _run(inputs, trace=False, **kw):
    nc = _build()
    context = np.ascontiguousarray(np.asarray(inputs["context"], dtype=np.float32))
    question = np.ascontiguousarray(np.asarray(inputs["question"], dtype=np.float32))
    wq = np.ascontiguousarray(np.asarray(inputs["w_question"], dtype=np.float32))
    wc = np.ascontiguousarray(np.asarray(inputs["w_context"], dtype=np.float32))
    wm = np.ascontiguousarray(np.asarray(inputs["w_multiple"], dtype=np.float32))
    in_maps = []
    for i in range(NCORES):
        sl = slice(i * BPC, (i + 1) * BPC)
        in_maps.append(
            {
                "context": context[sl],
                "question": question[sl],
                "w_question": wq,
                "w_context": wc,
                "w_multiple": wm,
            }
        )
    res = run_bass_kernel_spmd(
        nc, in_maps, core_ids=list(range(NCORES)), trace=trace, **kw
    )
    out = np.concatenate([res.results[i]["out"] for i in range(NCORES)], axis=0)
    return out, res


def kernel(**inputs):
    try:
        out, _ = _run(inputs, trace=False)
    except Exception:
        # transient device errors (e.g. a wedged core from a prior run)
        # usually clear on retry
        out, _ = _run(inputs, trace=False)
    return out

